# revision 4
# baseline (speedup 1.0000x reference)
"""Channel-attention (CAM) Bass kernel for TRN2, SPMD over 8 NeuronCores.

Computes, for each batch b:
    A   = inputs[b].reshape(HW, C)
    G   = A.T @ A                      (Gram, [C, C])
    S   = softmax(G, axis=-1)
    out = gamma * (A @ S) + A

Sharding: data-parallel over batch. 16 batches / 8 cores = 2 batches per core.

Per-core schedule (per batch):
  - A loaded as 8 DMA groups of [128, 4, 512] (1 MiB each).
  - Gram: 128 fp32r matmuls (lhsT = A-chunk column slice, rhs = A-chunk,
    N=512 moving dim -> full-rate fp32r) accumulated into 4 PSUM banks.
  - A^T built on-chip with PE transpose-mode matmuls (identity moving
    operand), drained PSUM->SBUF by DVE into AT [512, 4096].
  - Softmax: DVE row-max (negated) -> ScalarE Exp with accum_out row-sum
    -> DVE reciprocal -> DVE scale.
  - Attend: 128 fp32r matmuls, stationary = AT slice, moving = S tile,
    accumulated over the 4 c-chunks.
  - Epilogue: out = (psum * gamma) + A in one DVE scalar_tensor_tensor,
    then DMA to DRAM.
"""

import numpy as np

import concourse.bass as bass
import concourse.mybir as mybir
import concourse.tile as tile
from concourse import bacc
from concourse.bass import ds, ts
from concourse.masks import make_identity

P = 128
N_CORES = 8
B_TOTAL = 16
B_PER_CORE = B_TOTAL // N_CORES  # 2
H = 64
W = 64
HW = H * W          # 4096
C = 512
KO = HW // P        # 32 row chunks of A
M = C // P          # 4 channel chunks
NG = 8              # DMA groups
KPG = KO // NG      # chunks per group (4)

F32 = mybir.dt.float32
F32R = mybir.dt.float32r


def _build_kernel(tc, a_dram, gamma_dram, o_dram):
    nc = tc.nc
    from contextlib import ExitStack

    with ExitStack() as ctx:
        const_pool = ctx.enter_context(tc.tile_pool(name="const", bufs=1))
        a_pool = ctx.enter_context(tc.tile_pool(name="a", bufs=NG))
        at_pool = ctx.enter_context(tc.tile_pool(name="at", bufs=M))
        e_pool = ctx.enter_context(tc.tile_pool(name="e", bufs=M))
        s_pool = ctx.enter_context(tc.tile_pool(name="s", bufs=M))
        st_pool = ctx.enter_context(tc.tile_pool(name="st", bufs=16))
        o_pool = ctx.enter_context(tc.tile_pool(name="o", bufs=4))
        pg_pool = ctx.enter_context(tc.tile_pool(name="pg", bufs=M, space="PSUM"))
        pt_pool = ctx.enter_context(tc.tile_pool(name="pt", bufs=2, space="PSUM"))
        po_pool = ctx.enter_context(tc.tile_pool(name="po", bufs=2, space="PSUM"))

        ident = const_pool.tile([P, P], F32, tag="ident")
        make_identity(nc, ident)
        ident_r = const_pool.tile([P, P], F32R, tag="ident_r")
        nc.vector.tensor_copy(out=ident_r, in_=ident)
        gamma_sb = const_pool.tile([P, 1], F32, tag="gamma")
        nc.sync.dma_start(gamma_sb, gamma_dram)

        for b in range(B_PER_CORE):
            a_b = a_dram[b].rearrange("(ko p) c -> p ko c", p=P)
            a_g = []
            for g in range(NG):
                t = a_pool.tile([P, KPG, C], F32R, tag="a")
                nc.gpsimd.dma_start(t, a_b[:, ts(g, KPG), :])
                a_g.append(t)

            at = [at_pool.tile([P, HW], F32R, tag="at", name="at") for _ in range(M)]
            g_ps = [pg_pool.tile([P, C], F32, tag="pg", name="g_ps") for _ in range(M)]

            # Gram accumulation + on-chip transpose of A
            for g in range(NG):
                for j in range(KPG):
                    k = g * KPG + j
                    akr = a_g[g][:, j, :]
                    for m in range(M):
                        nc.tensor.matmul(
                            g_ps[m],
                            akr[:, ts(m, P)],
                            akr,
                            start=(k == 0),
                            stop=(k == KO - 1),
                        )
                for m in range(M):
                    pt = pt_pool.tile([P, C], F32R, tag="pt")
                    for j in range(KPG):
                        nc.tensor.transpose(
                            pt[:, ts(j, P)],
                            a_g[g][:, j, :][:, ts(m, P)],
                            ident_r,
                        )
                    nc.vector.tensor_copy(
                        out=at[m][:, ds(g * KPG * P, KPG * P)],
                        in_=pt,
                    )

            # Row softmax of G, one [128, 512] tile per channel chunk
            s_tiles = []
            for m in range(M):
                negmax = st_pool.tile([P, 1], F32, tag="stat")
                nc.vector.tensor_reduce(
                    negmax,
                    g_ps[m],
                    axis=mybir.AxisListType.X,
                    op=mybir.AluOpType.max,
                    negate=True,
                )
                e = e_pool.tile([P, C], F32, tag="e")
                d = st_pool.tile([P, 1], F32, tag="stat")
                nc.scalar.activation(
                    e,
                    g_ps[m],
                    mybir.ActivationFunctionType.Exp,
                    bias=negmax,
                    scale=1.0,
                    accum_out=d,
                )
                r = st_pool.tile([P, 1], F32, tag="stat")
                nc.vector.reciprocal(r, d)
                s = s_pool.tile([P, C], F32R, tag="s")
                nc.vector.tensor_scalar_mul(s, e, r)
                s_tiles.append(s)

            # Attend (A @ S) + residual epilogue
            for t_i in range(KO):
                o_ps = po_pool.tile([P, C], F32, tag="po")
                for m in range(M):
                    nc.tensor.matmul(
                        o_ps,
                        at[m][:, ts(t_i, P)],
                        s_tiles[m],
                        start=(m == 0),
                        stop=(m == M - 1),
                    )
                o_sb = o_pool.tile([P, C], F32, tag="o")
                nc.vector.scalar_tensor_tensor(
                    o_sb,
                    o_ps,
                    gamma_sb,
                    a_g[t_i // KPG][:, t_i % KPG, :],
                    op0=mybir.AluOpType.mult,
                    op1=mybir.AluOpType.add,
                )
                nc.sync.dma_start(o_dram[b][ts(t_i, P), :], o_sb)


_NC_CACHE = None


def build():
    global _NC_CACHE
    if _NC_CACHE is not None:
        return _NC_CACHE
    nc = bacc.Bacc(
        "TRN2",
        target_bir_lowering=False,
        debug=False,
        enable_asserts=False,
        num_devices=N_CORES,
    )
    a_dram = nc.dram_tensor("a", [B_PER_CORE, HW, C], F32, kind="ExternalInput").ap()
    gamma_dram = nc.dram_tensor("gamma", [P, 1], F32, kind="ExternalInput").ap()
    o_dram = nc.dram_tensor("o", [B_PER_CORE, HW, C], F32, kind="ExternalOutput").ap()
    with tile.TileContext(nc) as tc:
        _build_kernel(tc, a_dram, gamma_dram, o_dram)
    nc.compile()
    _NC_CACHE = nc
    return nc


def make_in_maps(inputs, gamma):
    x = np.ascontiguousarray(np.asarray(inputs, dtype=np.float32)).reshape(
        B_TOTAL, HW, C
    )
    gb = np.ascontiguousarray(
        np.broadcast_to(np.asarray(gamma, dtype=np.float32).reshape(1, 1), (P, 1))
    )
    return [
        {"a": x[i * B_PER_CORE : (i + 1) * B_PER_CORE], "gamma": gb}
        for i in range(N_CORES)
    ]


def run(inputs, gamma, trace=False, **kw):
    from concourse import bass_utils

    nc = build()
    in_maps = make_in_maps(inputs, gamma)
    res = bass_utils.run_bass_kernel_spmd(
        nc, in_maps, core_ids=list(range(N_CORES)), trace=trace, **kw
    )
    out = np.concatenate([r["o"] for r in res.results], axis=0)
    return out.reshape(B_TOTAL, H, W, C).astype(np.float32, copy=False), res


def kernel(inputs, gamma):
    out, _ = run(inputs, gamma, trace=False)
    return out
